# revision 19
# baseline (speedup 1.0000x reference)
"""Self-contained Trainium2 Bass kernel for nn_DiffuseMLP (GCN diffusion + MLP).

Contract: kernel(**inputs) takes FULL unsharded inputs (as in reference.setup_inputs())
and returns the FULL (128, 10) float32 output. Internally: shards dst rows of the
normalized adjacency across the 8 NeuronCores, runs one SPMD Bass kernel via
concourse.bass_utils.run_bass_kernel_spmd, and concatenates the per-core
(16, 10) batch slices of the output.

Strategy (dense SpMM): the per-edge dma_gather approach is descriptor-latency
bound on HW (~3.4ns/edge), so instead the host densifies the GCN-normalized
adjacency A^T into a per-core [N, DPC] bf16 slab (structure-only preprocessing:
deg norm + self loops folded in, duplicates summed). Per core:
  device: stream A^T in 128-src-row slabs ([128, DPC] bf16, 4KB/partition
  descriptors -> full DMA rate); PE: for each src block sb, for each dst block
  db: acc_db[128d, B] += AT[sb,db]^T @ xT[sb] accumulated over all 128 sb in
  PSUM; evict acc_db -> bf16 hTg; h1 += hTg^T @ W1T_db accumulated in PSUM.
  tail: ReduceScatter h1 over batch (each core keeps 16 rows); local MLP
  (relu+b1, W2, relu+b2, Wfc, +bfc) -> out slice [16, 10].
"""
import sys, os
for _p in ("/opt/trn_rl_repo", os.path.expanduser("~/.axon_site/_ro/trn_rl_repo")):
    if os.path.isdir(_p) and _p not in sys.path:
        sys.path.insert(0, _p)

import numpy as np
import ml_dtypes
import scipy.sparse as sp
import concourse.bass as bass
import concourse.tile as tile
import concourse.mybir as mybir
from concourse.bass_utils import run_bass_kernel_spmd
from concourse.vector_clock import ScopedClock
from concourse.masks import make_identity
from concourse import library_config

BF16 = mybir.dt.bfloat16
F32 = mybir.dt.float32
I16 = mybir.dt.int16
np_bf16 = ml_dtypes.bfloat16

B, N = 128, 16384
H, COUT = 512, 10
NCORES = 8
DPC = N // NCORES        # dsts per core (2048)
SB = N // 128            # src blocks (128)
DB = DPC // 128          # dst blocks per core (16)
BPC = B // NCORES        # batch rows per core after ReduceScatter


# --------------------------------------------------------------------------
# Workarounds for this walrus build: it rejects >1 sync wait per instruction.
# --------------------------------------------------------------------------

def _patched_drain_and_barrier(self, tick_clock, wait_clock):
    nc = self.nc
    drain_inst = nc.sync.drain()
    wait_clock.add_sem_waits(drain_inst.ins, ScopedClock({None: tick_clock.global_clock}))
    si = drain_inst.ins.sync_info
    waits = list(si.on_wait or []) if si else []
    if len(waits) > 1:
        bb = nc.cur_bb.bb
        insts = bb.instructions
        assert insts[-1].name == drain_inst.ins.name
        popped = insts.pop()
        si.on_wait = [waits[-1]]
        for w in waits[:-1]:
            nop = nc.sync.nop(nofuse=True)
            nsi = nop.ins.sync_info
            if nsi is None:
                nop.ins.sync_info = mybir.SyncInfo(on_wait=[w], on_update=[])
            else:
                nsi.on_wait = [w]
        bb.add_instruction(popped)
    nc.all_engine_barrier()
    assert self.sems is not None
    popped_p = nc._tile_sem_poison_stack.pop()
    assert popped_p is self._sem_poison
    nc.clear_and_free_semaphores(list(self.sems.allocated().values()))
    nc.all_engine_barrier()


tile.TileContext._drain_and_barrier = _patched_drain_and_barrier


def legalize_waits(nc, max_waits=1):
    n_split = 0
    for fn in nc.m.functions:
        for bb in fn.blocks:
            insts = list(bb.instructions)
            out = []
            for inst in insts:
                si = inst.sync_info
                waits = list(si.on_wait or []) if si else []
                if len(waits) > max_waits:
                    n_split += 1
                    for i in range(0, len(waits) - max_waits, max_waits):
                        chunk = waits[i:i + max_waits]
                        nop = mybir.InstNoOp(
                            name=f"waitnop_{inst.name}_{i}", ins=[], outs=[],
                            sync_info=mybir.SyncInfo(on_wait=chunk, on_update=[]),
                        )
                        nop.engine = inst.engine
                        nc.register_instruction(nop, overwrite=True)
                        out.append(nop)
                    si.on_wait = waits[len(waits) - max_waits:]
                out.append(inst)
            if len(out) != len(insts):
                bb.instructions.clear()
                for i in out:
                    bb.add_instruction(i)
    return n_split


# --------------------------------------------------------------------------
# Host-side sharding / layout prep
# --------------------------------------------------------------------------

def make_inmaps(inputs):
    x = np.asarray(inputs["x"], np.float32)
    src = np.asarray(inputs["edge_index"][0]).astype(np.int64)
    dst = np.asarray(inputs["edge_index"][1]).astype(np.int64)
    w = np.asarray(inputs["edge_weight"], np.float32)
    loop = np.arange(N, dtype=np.int64)
    src_f = np.concatenate([src, loop])
    dst_f = np.concatenate([dst, loop])
    w_f = np.concatenate([w, np.ones(N, np.float32)])

    deg = np.bincount(dst_f, weights=w_f.astype(np.float64), minlength=N)
    dinv = (1.0 / np.sqrt(np.maximum(deg, 1e-12))).astype(np.float32)
    wn = (dinv[src_f] * w_f * dinv[dst_f]).astype(np.float32)

    # xq: x^T in the SBUF layout [p, sb, b] (node n = sb*128 + p), contiguous
    xq = np.ascontiguousarray(
        x.T.reshape(SB, 128, B).transpose(1, 0, 2)).astype(np_bf16)

    W1T = np.ascontiguousarray(np.asarray(inputs["W1"], np.float32).T).astype(np_bf16)
    W2T = np.ascontiguousarray(np.asarray(inputs["W2"], np.float32).T).astype(np_bf16)
    WfcT = np.ascontiguousarray(np.asarray(inputs["Wfc"], np.float32).T).astype(np_bf16)
    b1r = np.ascontiguousarray(np.asarray(inputs["b1"], np.float32).reshape(H // 128, 128).T)
    b2r = np.ascontiguousarray(np.asarray(inputs["b2"], np.float32).reshape(H // 128, 128).T)
    bfc16 = np.tile(np.asarray(inputs["bfc"], np.float32)[None, :], (128, 1))

    in_maps = []
    for c in range(NCORES):
        m = (dst_f >= c * DPC) & (dst_f < (c + 1) * DPC)
        # dense A^T slab for this core: [N src, DPC dst], duplicates summed
        at = sp.coo_matrix(
            (wn[m], (src_f[m], dst_f[m] - c * DPC)), shape=(N, DPC)).toarray()
        # layout [p, db, sb, dl]: src node n = sb*128 + p, dst d = db*128 + dl;
        # per (p, db) contiguous 32KB -> one 4MB full-rate DMA per dst block
        atq = np.ascontiguousarray(
            at.reshape(SB, 128, DB, 128).transpose(1, 2, 0, 3)).astype(np_bf16)
        in_maps.append({
            "atq": atq,
            "xq": xq,
            "w1t": np.ascontiguousarray(W1T[c * DPC:(c + 1) * DPC]),
            "w2t": W2T, "wfct": WfcT,
            "b1r": b1r, "b2r": b2r, "bfc16": bfc16,
        })
    return (in_maps,)


def build(ABUFS=4, coll="rs"):
    nc = bass.Bass()

    # ---- I/O ----
    atq_d = nc.declare_dram_parameter("atq", [128, DB, SB, 128], BF16, isOutput=False)
    xq_d = nc.declare_dram_parameter("xq", [128, SB, B], BF16, isOutput=False)
    w1t_d = nc.declare_dram_parameter("w1t", [DPC, H], BF16, isOutput=False)
    w2t_d = nc.declare_dram_parameter("w2t", [H, H], BF16, isOutput=False)
    wfct_d = nc.declare_dram_parameter("wfct", [H, COUT], BF16, isOutput=False)
    b1_d = nc.declare_dram_parameter("b1r", [128, H // 128], F32, isOutput=False)
    b2_d = nc.declare_dram_parameter("b2r", [128, H // 128], F32, isOutput=False)
    bfc_d = nc.declare_dram_parameter("bfc16", [128, COUT], F32, isOutput=False)
    RROWS = B if coll == "ar" else BPC  # rows handled by this core's MLP tail
    out_d = nc.declare_dram_parameter("out", [RROWS, COUT], F32, isOutput=True)

    CDT = F32 if coll == "ar" else BF16  # h1 collective dtype
    h1_bounce = nc.dram_tensor("h1_bounce", [B, H], CDT)
    if coll == "ar":
        h1_red = nc.dram_tensor("h1_red", [B, H], CDT, addr_space="Shared")
    else:
        h1_red = nc.dram_tensor("h1_red", [BPC, H], CDT)

    with nc.Block() as _blk:
        @_blk.gpsimd
        def _(gp):
            gp.load_library(library_config.mlp)

    with tile.TileContext(nc) as tc:
        with tc.tile_pool(name="const", bufs=1) as constp, \
             tc.tile_pool(name="sb", bufs=1) as sbp, \
             tc.tile_pool(name="at", bufs=ABUFS) as atp, \
             tc.tile_pool(name="hT", bufs=2) as hTp, \
             tc.tile_pool(name="psH", bufs=1, space="PSUM") as psH, \
             tc.tile_pool(name="psA", bufs=2, space="PSUM") as psA, \
             tc.tile_pool(name="psM", bufs=1, space="PSUM") as psM, \
             tc.tile_pool(name="psT", bufs=1, space="PSUM") as psT:

            ident = constp.tile([128, 128], F32)
            make_identity(nc, ident[:])
            identb = constp.tile([128, 128], BF16)
            make_identity(nc, identb[:])

            # ---- persistent loads (scalar HWDGE ring: overlaps the sync-ring
            # A-slab stream; xq halves so the first matmuls start sooner) ----
            xq_t = sbp.tile([128, SB, B], BF16)
            nc.scalar.dma_start(xq_t[:, :SB // 2, :], xq_d[:, :SB // 2, :])
            nc.scalar.dma_start(xq_t[:, SB // 2:, :], xq_d[:, SB // 2:, :])
            w1t_t = sbp.tile([128, DPC // 128, H], BF16)
            nc.scalar.dma_start(w1t_t[:], w1t_d[:].rearrange("(t p) h -> p t h", p=128))
            w2t_t = sbp.tile([128, H // 128, H], BF16)
            nc.scalar.dma_start(w2t_t[:], w2t_d[:].rearrange("(t p) h -> p t h", p=128))
            wfct_t = sbp.tile([128, H // 128, COUT], BF16)
            nc.scalar.dma_start(wfct_t[:], wfct_d[:].rearrange("(t p) h -> p t h", p=128))
            b1_t = sbp.tile([128, H // 128], F32)
            nc.scalar.dma_start(b1_t[:], b1_d[:])
            b2_t = sbp.tile([128, H // 128], F32)
            nc.scalar.dma_start(b2_t[:], b2_d[:])
            bfc_t = sbp.tile([128, COUT], F32)
            nc.scalar.dma_start(bfc_t[:], bfc_d[:])

            # ---- dense diffusion, dst-block major ----
            # per db: one 4MB slab DMA, acc[128d, B] += AT[sb]^T @ xT[sb] over
            # all 128 src blocks, evict, h1[B, H] += hTg^T @ W1T_db
            h1ps = psH.tile([128, H], F32, space="PSUM", tag="h1")
            for db in range(DB):
                at_t = atp.tile([128, SB, 128], BF16, tag="at")
                # alternate slabs across the two HWDGE rings (sync/scalar) so
                # the SDMA engines always have two transfer streams in flight;
                # split the last slab so its PE burst starts before the full
                # 4MB lands (shaves tail-start latency)
                eng = nc.sync if db % 2 == 0 else nc.scalar
                nsplit = 4 if db == DB - 1 else 1
                step = SB // nsplit
                for s0 in range(0, SB, step):
                    eng.dma_start(at_t[:, s0:s0 + step, :],
                                  atq_d[:, db, s0:s0 + step, :])
                acc = psA.tile([128, B], F32, space="PSUM", tag="acc")
                for sb in range(SB):
                    nc.tensor.matmul(out=acc[:],
                                     lhsT=at_t[:, sb, :],
                                     rhs=xq_t[:, sb, :],
                                     start=(sb == 0), stop=(sb == SB - 1))
                hTg = hTp.tile([128, B], BF16, tag="hT")
                nc.scalar.activation(out=hTg[:], in_=acc[:],
                                     func=mybir.ActivationFunctionType.Copy)
                nc.tensor.matmul(out=h1ps[:], lhsT=hTg[:], rhs=w1t_t[:, db, :],
                                 start=(db == 0), stop=(db == DB - 1))

            # ---- h1 partial -> collective over cores ----
            h1sb = sbp.tile([128, H], CDT)
            nc.vector.tensor_copy(out=h1sb[:], in_=h1ps[:])
            nc.sync.dma_start(h1_bounce[:], h1sb[:])
            if coll == "rs":
                nc.gpsimd.collective_compute(
                    "ReduceScatter", mybir.AluOpType.add,
                    replica_groups=[list(range(NCORES))],
                    ins=[h1_bounce[:]], outs=[h1_red[:]],
                )
            else:
                nc.gpsimd.collective_compute(
                    "AllReduce", mybir.AluOpType.add,
                    replica_groups=[list(range(NCORES))],
                    ins=[h1_bounce[:]], outs=[h1_red[:]],
                )
            h1r = sbp.tile([RROWS, H], CDT)
            nc.sync.dma_start(h1r[:], h1_red[:])

            # ---- local MLP tail on RROWS batch rows ----
            KH = H // 128  # 4
            h1T = sbp.tile([128, KH, RROWS], BF16)
            for t in range(KH):
                tp = psT.tile([128, RROWS], CDT, space="PSUM", tag="tpose")
                nc.tensor.transpose(out=tp[:], in_=h1r[:, t * 128:(t + 1) * 128],
                                    identity=(ident if coll == "ar" else identb)[:RROWS, :RROWS])
                nc.scalar.activation(out=h1T[:, t, :], in_=tp[:],
                                     func=mybir.ActivationFunctionType.Relu,
                                     bias=b1_t[:, t:t + 1])
            h2ps = psM.tile([RROWS, H], F32, space="PSUM", tag="mlp2")
            for t in range(KH):
                nc.tensor.matmul(out=h2ps[:], lhsT=h1T[:, t, :], rhs=w2t_t[:, t, :],
                                 start=(t == 0), stop=(t == KH - 1))
            h2sb = sbp.tile([RROWS, H], F32)
            nc.vector.tensor_copy(out=h2sb[:], in_=h2ps[:])
            h2T = sbp.tile([128, KH, RROWS], BF16)
            for t in range(KH):
                tp = psT.tile([128, RROWS], F32, space="PSUM", tag="tpose")
                nc.tensor.transpose(out=tp[:], in_=h2sb[:, t * 128:(t + 1) * 128],
                                    identity=ident[:RROWS, :RROWS])
                nc.scalar.activation(out=h2T[:, t, :], in_=tp[:],
                                     func=mybir.ActivationFunctionType.Relu,
                                     bias=b2_t[:, t:t + 1])
            outps = psM.tile([RROWS, COUT], F32, space="PSUM", tag="mlp3")
            for t in range(KH):
                nc.tensor.matmul(out=outps[:], lhsT=h2T[:, t, :], rhs=wfct_t[:, t, :],
                                 start=(t == 0), stop=(t == KH - 1))
            out_t = sbp.tile([RROWS, COUT], F32)
            nc.vector.tensor_add(out=out_t[:], in0=outps[:], in1=bfc_t[:RROWS, :])
            nc.sync.dma_start(out_d[:], out_t[:])

    return nc


# --------------------------------------------------------------------------
# Public entry point
# --------------------------------------------------------------------------

_BUILD_CACHE = {}


def _get_built(**kw):
    key = tuple(sorted(kw.items()))
    if key not in _BUILD_CACHE:
        nc = build(**kw)
        legalize_waits(nc, max_waits=1)
        mybir.codegen_inst_isa_subclasses(nc)
        _BUILD_CACHE[key] = nc
    return _BUILD_CACHE[key]


def kernel(**inputs) -> np.ndarray:
    (in_maps,) = make_inmaps(inputs)
    nc = _get_built()
    res = run_bass_kernel_spmd(nc, in_maps, list(range(NCORES)))
    if res.results[0]["out"].shape[0] == B:
        return np.asarray(res.results[0]["out"], np.float32)
    return np.concatenate(
        [np.asarray(res.results[c]["out"], np.float32) for c in range(NCORES)], axis=0)
